# revision 1
# baseline (speedup 1.0000x reference)
"""Trainium2 Bass kernel for nn_CornerActivationB.

Math: the reference expands a binary corner table [G, 4, D] to a ternary
grid [G, 9, D] via midpoint averaging, then does piecewise-bilinear
interpolation on the 3x3 grid. Midpoints are exact averages, so the
piecewise-bilinear interpolant of those samples IS the bilinear function
of the 4 binary corners:

    out[b, g, d] = c0[g,d] + u0*c1[g,d] + u1*c2[g,d] + u0*u1*c3[g,d]

with u = clip(x, -1, 1) and c* fixed +-0.25-multiples of corner sums
(exact in bf16). Verified vs reference: fp32 absmax 1.8e-7.

Kernel structure (per core, batch-sharded 8192/8 = 1024 rows):
  - q[b, g*4+c] = [1, u0, u1, u0*u1] in bf16  (DVE elementwise)
  - PE-transpose 128-column chunks of q -> qT (contraction on partitions)
  - block-diag matmul: out[128b, 512] = qT.T-contract @ W[128, 512]
    where W chunk holds 32 groups' [4 x 16] coefficient blocks
  - evict PSUM -> SBUF (DVE/ACT split), SWDGE DMA 2 MiB blocks to HBM

Engine assignment keeps semaphore-wait fan-in per instruction low (the
ISA has one wait slot; legalize_waits() splits overflow onto NoOps): q
has a single producer proc (DVE), qT eviction is all-ACT, out evictions
alternate DVE/ACT per EVICT_GROUP so each out buffer has one producer.
Input/weight DMAs ride the SP HWDGE ring; output DMAs are SWDGE
(single completion semaphore per DMA).
"""

import numpy as np
import ml_dtypes
from contextlib import ExitStack

import bass_rust
import concourse.bass as bass
import concourse.mybir as mybir
import concourse.tile as tile
from concourse import masks
from concourse.bass_utils import run_bass_kernel_spmd

BATCH = 8192
GROUPS = 512
ARITY = 2
OUT_DIM = 16
N_CORES = 8
B_LOC = BATCH // N_CORES          # 1024 rows per core
P = 128                           # partition tile
N_TILES = B_LOC // P              # 8 batch tiles per core
GPC = 32                          # groups per contraction chunk (32*4 = 128 = K)
N_CHUNKS = GROUPS // GPC          # 16
CHUNK_COLS = GPC * OUT_DIM        # 512 output cols per chunk (one PSUM bank)
EVICT_GROUP = 8                   # psum chunks per output DMA (2 MiB)
QT_PACK = 4                       # transposes packed per qt PSUM bank

_BF16 = mybir.dt.bfloat16
_F32 = mybir.dt.float32


def legalize_waits(nc: bass.Bass, cap: int = 1) -> None:
    """Split instructions carrying more than `cap` semaphore waits.

    Hardware instructions have a fixed number of sync-wait slots and walrus
    rejects overflow ("Too many sync wait commands"). Tile's scheduler can
    emit 3+ waits on one instruction; move the excess onto NoOp instructions
    inserted immediately before it on the same engine — semantically
    identical (same program point on the same sequencer), so no deadlock or
    reordering risk.
    """
    n = 0
    for f in nc.m.functions:
        for bb in f.blocks:
            insts = bb.instructions
            out = []
            changed = False
            for ins in insts:
                si = ins.sync_info
                if si is not None and len(si.on_wait) > cap:
                    waits = list(si.on_wait)
                    keep, extra = waits[:cap], waits[cap:]
                    while extra:
                        chunk, extra = extra[:cap], extra[cap:]
                        nop = mybir.InstNoOp(name=f"wait-legalize-{n}")
                        n += 1
                        nop.engine = ins.engine
                        nop.sync_info = bass_rust.SyncInfo(
                            on_wait=chunk, on_update=[]
                        )
                        out.append(nop)
                    ins.sync_info = bass_rust.SyncInfo(
                        on_wait=keep, on_update=si.on_update
                    )
                    changed = True
                out.append(ins)
            if changed:
                bb.instructions = out


def build_nc(legalize: bool = True) -> bass.Bass:
    nc = bass.Bass()
    x = nc.declare_dram_parameter("x", [B_LOC, GROUPS * ARITY], _F32, isOutput=False)
    w = nc.declare_dram_parameter("w", [P, N_CHUNKS * CHUNK_COLS], _BF16, isOutput=False)
    out = nc.declare_dram_parameter("out", [B_LOC, GROUPS * OUT_DIM], _F32, isOutput=True)

    with tile.TileContext(nc) as tc, ExitStack() as ctx:
        singles = ctx.enter_context(tc.tile_pool(name="singles", bufs=1))
        xp = ctx.enter_context(tc.tile_pool(name="xp", bufs=8))
        qp = ctx.enter_context(tc.tile_pool(name="qp", bufs=2))
        qtp = ctx.enter_context(tc.tile_pool(name="qtp", bufs=2, space="PSUM"))
        qts = ctx.enter_context(tc.tile_pool(name="qts", bufs=2))
        outp = ctx.enter_context(tc.tile_pool(name="outp", bufs=6, space="PSUM"))
        outs = ctx.enter_context(tc.tile_pool(name="outs", bufs=6))

        # issue the first x tile (SWDGE) and the W load (SP HWDGE) before
        # building the identity so DMA starts at t=0 on both paths
        x0_t = xp.tile([P, GROUPS, ARITY], _F32, tag="xt")
        nc.sync.dma_start(
            out=x0_t[:].rearrange("p g a -> p (g a)"), in_=x[0:P, :]
        )
        w_sb = singles.tile([P, N_CHUNKS * CHUNK_COLS], _BF16)
        nc.sync.dma_start(out=w_sb[:], in_=w[:])

        ident = singles.tile([P, P], _BF16)
        masks.make_identity(nc, ident[:])

        for it in range(N_TILES):
            if it == 0:
                x_t = x0_t
            else:
                x_t = xp.tile([P, GROUPS, ARITY], _F32, tag="xt")
                nc.sync.dma_start(
                    out=x_t[:].rearrange("p g a -> p (g a)"),
                    in_=x[it * P:(it + 1) * P, :],
                )

            # all q-prep on DVE so q has a single producer proc
            q_t = qp.tile([P, GROUPS, 4], _BF16)
            nc.vector.memset(q_t[:, :, 0], 1.0)
            nc.vector.tensor_scalar(
                out=q_t[:, :, 1], in0=x_t[:, :, 0],
                scalar1=1.0, scalar2=-1.0,
                op0=mybir.AluOpType.min, op1=mybir.AluOpType.max,
            )
            nc.vector.tensor_scalar(
                out=q_t[:, :, 2], in0=x_t[:, :, 1],
                scalar1=1.0, scalar2=-1.0,
                op0=mybir.AluOpType.min, op1=mybir.AluOpType.max,
            )
            nc.vector.tensor_tensor(
                out=q_t[:, :, 3], in0=q_t[:, :, 1], in1=q_t[:, :, 2],
                op=mybir.AluOpType.mult,
            )
            qf = q_t[:].rearrange("p g c -> p (g c)")   # [128, 2048]

            out_sb = None
            qt_sb = None
            for j in range(N_CHUNKS):
                k = j % QT_PACK
                if k == 0:
                    # pack 4 transposes into one PSUM bank, evict with a
                    # single [128, 512] bf16 copy (all qT evictions on ACT)
                    qt_ps = qtp.tile([P, QT_PACK, P], _BF16)
                    for kk in range(QT_PACK):
                        jj = j + kk
                        nc.tensor.transpose(
                            qt_ps[:, kk, :], qf[:, jj * P:(jj + 1) * P], ident[:]
                        )
                    qt_sb = qts.tile([P, QT_PACK, P], _BF16)
                    nc.scalar.copy(
                        qt_sb[:].rearrange("p k c -> p (k c)"),
                        qt_ps[:].rearrange("p k c -> p (k c)"),
                    )

                o_ps = outp.tile([P, CHUNK_COLS], _F32)
                nc.tensor.matmul(
                    o_ps[:], lhsT=qt_sb[:, k, :],
                    rhs=w_sb[:, j * CHUNK_COLS:(j + 1) * CHUNK_COLS],
                    start=True, stop=True,
                )

                m = j % EVICT_GROUP
                if m == 0:
                    out_sb = outs.tile([P, EVICT_GROUP * CHUNK_COLS], _F32)
                # alternate evicting engine per DMA group: each out_sb buffer
                # (and its DMA) depends on exactly one compute engine
                dst = out_sb[:, m * CHUNK_COLS:(m + 1) * CHUNK_COLS]
                if (j // EVICT_GROUP) % 2 == 0:
                    nc.vector.tensor_copy(dst, o_ps[:])
                else:
                    nc.scalar.copy(dst, o_ps[:])
                if m == EVICT_GROUP - 1:
                    c0 = (j + 1 - EVICT_GROUP) * CHUNK_COLS
                    nc.gpsimd.dma_start(
                        out=out[it * P:(it + 1) * P, c0:c0 + EVICT_GROUP * CHUNK_COLS],
                        in_=out_sb[:],
                    )
    if legalize:
        legalize_waits(nc)
    return nc


def make_w_host(params: np.ndarray) -> np.ndarray:
    """Coefficient matrix: [P, N_CHUNKS*512] bf16, w_host[p, t*512+n] = Wm[t, p, n]
    where Wm[t, gl*4+c, gl*16+d] = C[32t+gl, c, d]."""
    p4 = np.asarray(params, dtype=np.float32)            # [G, 4, D]
    p00, p01, p10, p11 = p4[:, 0], p4[:, 1], p4[:, 2], p4[:, 3]
    c = np.stack(
        [
            (p00 + p01 + p10 + p11) * 0.25,
            (p10 + p11 - p00 - p01) * 0.25,
            (p01 + p11 - p00 - p10) * 0.25,
            (p00 + p11 - p01 - p10) * 0.25,
        ],
        axis=1,
    )                                                    # [G, 4, D]
    wm = np.zeros((N_CHUNKS, P, CHUNK_COLS), np.float32)
    cr = c.reshape(N_CHUNKS, GPC, 4, OUT_DIM)
    for gl in range(GPC):
        wm[:, gl * 4:(gl + 1) * 4, gl * OUT_DIM:(gl + 1) * OUT_DIM] = cr[:, gl]
    w_host = np.ascontiguousarray(wm.transpose(1, 0, 2).reshape(P, N_CHUNKS * CHUNK_COLS))
    return w_host.astype(ml_dtypes.bfloat16)


_NC_CACHE = {}


def kernel(X: np.ndarray, params: np.ndarray) -> np.ndarray:
    X = np.ascontiguousarray(np.asarray(X, dtype=np.float32))
    assert X.shape == (BATCH, GROUPS * ARITY)
    w_host = make_w_host(params)

    if "nc" not in _NC_CACHE:
        _NC_CACHE["nc"] = build_nc()
    nc = _NC_CACHE["nc"]

    in_maps = [
        {"x": X[i * B_LOC:(i + 1) * B_LOC], "w": w_host} for i in range(N_CORES)
    ]
    res = run_bass_kernel_spmd(nc, in_maps, core_ids=list(range(N_CORES)))
    return np.concatenate(
        [np.asarray(res.results[i]["out"]) for i in range(N_CORES)], axis=0
    )



# revision 2
# speedup vs baseline: 1.5025x; 1.5025x over previous
"""Trainium2 Bass kernel for nn_CornerActivationB.

Math: the reference expands a binary corner table [G, 4, D] to a ternary grid
[G, 9, D] via midpoint averaging, then does piecewise-bilinear interpolation on
the 3x3 grid. Midpoints are exact averages, so the interpolant equals the
bilinear function of the 4 binary corners:

    out[b, g, d] = c0[g,d] + u0*c1[g,d] + u1*c2[g,d] + u0*u1*c3[g,d]

with u = clip(x, -1, 1) and c* fixed +-0.25-multiples of corner sums (exact in
bf16).

Kernel structure (per core, batch-sharded 8192/8 = 1024 rows, 8 tiles of 128):
  - q[b, g*4+c] = [1, u0, u1, u0*u1] bf16; clips on DVE, memset/mult on GPSIMD
  - PE-transpose 128-col chunks of q (8 packed per PSUM bank, ACT evicts)
  - block-diag matmul per chunk: out[128b, 512] = qT.T @ W[128, 512] where the
    W chunk holds 32 groups' [4 x 16] coefficient blocks
  - PSUM fp32 -> SBUF bf16 cast-evictions alternate DVE/ACT per chunk
  - SWDGE DMAs write bf16 rows to HBM; host upcasts to fp32 (bf16 quantization
    adds ~1e-3 rel error against a 2e-2 tolerance)

Measured on-HW (neuron-profile): ~86.6us vs ~125.6us for the fp32-out
baseline. Output DMA traffic halves to 16.8MB/core; kernel is PE/ACT-bound.
The first tile's output goes out in (2,2,4,8)-chunk groups so the write stream
starts early; mid tiles use one full-row DMA (16KB descriptors, balanced
round-robin across all 16 DMA engines); the last tile splits in two to shorten
the tail. x/W ride the SP HWDGE ring, W split so chunk 0 lands first.
"""

import numpy as np
import ml_dtypes
from contextlib import ExitStack

import bass_rust
import concourse.bass as bass
import concourse.mybir as mybir
import concourse.tile as tile
from concourse import masks
from concourse.bass_utils import run_bass_kernel_spmd

BATCH = 8192
GROUPS = 512
ARITY = 2
OUT_DIM = 16
N_CORES = 8
B_LOC = BATCH // N_CORES          # 1024 rows per core
P = 128                           # partition tile
N_TILES = B_LOC // P              # 8 batch tiles per core
GPC = 32                          # groups per contraction chunk (32*4 = 128 = K)
N_CHUNKS = GROUPS // GPC          # 16
CHUNK_COLS = GPC * OUT_DIM        # 512 output cols per chunk (one PSUM bank)
QT_PACK = 8                       # transposes packed per qt PSUM bank
FIRST_GROUPS = (2, 2, 4, 8)       # tile-0 output DMA chunk groups (early start)
LAST_GROUPS = (8, 8)              # last-tile groups (shorter tail)
W_SPLITS = (1, 3, 12)             # W load split so chunk 0 arrives first

_BF16 = mybir.dt.bfloat16
_F32 = mybir.dt.float32


def legalize_waits(nc: bass.Bass, cap: int = 1) -> None:
    """Split instructions carrying more than `cap` semaphore waits.

    Hardware instructions have a fixed number of sync-wait slots and walrus
    rejects overflow ("Too many sync wait commands"). Tile's scheduler can
    emit 3+ waits on one instruction; move the excess onto NoOp instructions
    inserted immediately before it on the same engine — semantically
    identical (same program point on the same sequencer).
    """
    n = 0
    for f in nc.m.functions:
        for bb in f.blocks:
            insts = bb.instructions
            out = []
            changed = False
            for ins in insts:
                si = ins.sync_info
                if si is not None and len(si.on_wait) > cap:
                    waits = list(si.on_wait)
                    keep, extra = waits[:cap], waits[cap:]
                    while extra:
                        chunk, extra = extra[:cap], extra[cap:]
                        nop = mybir.InstNoOp(name=f"wait-legalize-{n}")
                        n += 1
                        nop.engine = ins.engine
                        nop.sync_info = bass_rust.SyncInfo(
                            on_wait=chunk, on_update=[]
                        )
                        out.append(nop)
                    ins.sync_info = bass_rust.SyncInfo(
                        on_wait=keep, on_update=si.on_update
                    )
                    changed = True
                out.append(ins)
            if changed:
                bb.instructions = out


def build_nc(legalize: bool = True) -> bass.Bass:
    nc = bass.Bass()
    x = nc.declare_dram_parameter("x", [B_LOC, GROUPS * ARITY], _F32, isOutput=False)
    w = nc.declare_dram_parameter("w", [P, N_CHUNKS * CHUNK_COLS], _BF16, isOutput=False)
    out = nc.declare_dram_parameter("out", [B_LOC, GROUPS * OUT_DIM], _BF16, isOutput=True)

    with tile.TileContext(nc) as tc, ExitStack() as ctx:
        singles = ctx.enter_context(tc.tile_pool(name="singles", bufs=1))
        xp = ctx.enter_context(tc.tile_pool(name="xp", bufs=4))
        qp = ctx.enter_context(tc.tile_pool(name="qp", bufs=2))
        qtp = ctx.enter_context(tc.tile_pool(name="qtp", bufs=2, space="PSUM"))
        qts = ctx.enter_context(tc.tile_pool(name="qts", bufs=2))
        outp = ctx.enter_context(tc.tile_pool(name="outp", bufs=6, space="PSUM"))
        outs = ctx.enter_context(tc.tile_pool(name="outs", bufs=3))

        # x tile 0 first, then W in chunk-splits so chunk-0 matmuls start early
        x0_t = xp.tile([P, GROUPS, ARITY], _F32, tag="xt")
        nc.sync.dma_start(out=x0_t[:].rearrange("p g a -> p (g a)"), in_=x[0:P, :])
        w_sb = singles.tile([P, N_CHUNKS * CHUNK_COLS], _BF16)
        c = 0
        for ws in W_SPLITS:
            nc.sync.dma_start(
                out=w_sb[:, c * CHUNK_COLS:(c + ws) * CHUNK_COLS],
                in_=w[:, c * CHUNK_COLS:(c + ws) * CHUNK_COLS],
            )
            c += ws

        ident = singles.tile([P, P], _BF16)
        masks.make_identity(nc, ident[:])

        for it in range(N_TILES):
            if it == 0:
                x_t = x0_t
            else:
                x_t = xp.tile([P, GROUPS, ARITY], _F32, tag="xt")
                nc.sync.dma_start(
                    out=x_t[:].rearrange("p g a -> p (g a)"),
                    in_=x[it * P:(it + 1) * P, :],
                )

            # q-prep: clips on DVE, memset + product on GPSIMD (keeps DVE
            # capacity for the PSUM cast-evictions)
            q_t = qp.tile([P, GROUPS, 4], _BF16)
            nc.gpsimd.memset(q_t[:, :, 0], 1.0)
            nc.vector.tensor_scalar(
                out=q_t[:, :, 1], in0=x_t[:, :, 0],
                scalar1=1.0, scalar2=-1.0,
                op0=mybir.AluOpType.min, op1=mybir.AluOpType.max,
            )
            nc.vector.tensor_scalar(
                out=q_t[:, :, 2], in0=x_t[:, :, 1],
                scalar1=1.0, scalar2=-1.0,
                op0=mybir.AluOpType.min, op1=mybir.AluOpType.max,
            )
            nc.gpsimd.tensor_tensor(
                out=q_t[:, :, 3], in0=q_t[:, :, 1], in1=q_t[:, :, 2],
                op=mybir.AluOpType.mult,
            )
            qf = q_t[:].rearrange("p g c -> p (g c)")   # [128, 2048]

            if it == 0:
                groups = list(FIRST_GROUPS)
            elif it == N_TILES - 1:
                groups = list(LAST_GROUPS)
            else:
                groups = [N_CHUNKS]
            gmap = {}
            gs = 0
            for gl in groups:
                for j in range(gs, gs + gl):
                    gmap[j] = (gs, gl)
                gs += gl

            out_sb = outs.tile([P, N_CHUNKS * CHUNK_COLS], _BF16)
            qt_sb = None
            for j in range(N_CHUNKS):
                k = j % QT_PACK
                if k == 0:
                    # pack 8 transposes into one PSUM bank, evict with a
                    # single [128, 1024] bf16 copy on ACT
                    qt_ps = qtp.tile([P, QT_PACK, P], _BF16)
                    for kk in range(QT_PACK):
                        jj = j + kk
                        nc.tensor.transpose(
                            qt_ps[:, kk, :], qf[:, jj * P:(jj + 1) * P], ident[:]
                        )
                    qt_sb = qts.tile([P, QT_PACK, P], _BF16)
                    nc.scalar.copy(
                        qt_sb[:].rearrange("p k c -> p (k c)"),
                        qt_ps[:].rearrange("p k c -> p (k c)"),
                    )

                o_ps = outp.tile([P, CHUNK_COLS], _F32)
                nc.tensor.matmul(
                    o_ps[:], lhsT=qt_sb[:, k, :],
                    rhs=w_sb[:, j * CHUNK_COLS:(j + 1) * CHUNK_COLS],
                    start=True, stop=True,
                )

                # cast-evict PSUM fp32 -> SBUF bf16, alternating DVE/ACT
                dst = out_sb[:, j * CHUNK_COLS:(j + 1) * CHUNK_COLS]
                if j % 2 == 0:
                    nc.vector.tensor_copy(dst, o_ps[:])
                else:
                    nc.scalar.copy(dst, o_ps[:])

                g0, gl = gmap[j]
                if j == g0 + gl - 1:
                    c0 = g0 * CHUNK_COLS
                    nc.gpsimd.dma_start(
                        out=out[it * P:(it + 1) * P, c0:c0 + gl * CHUNK_COLS],
                        in_=out_sb[:, c0:c0 + gl * CHUNK_COLS],
                    )
    if legalize:
        legalize_waits(nc)
    return nc


def make_w_host(params: np.ndarray) -> np.ndarray:
    """Coefficient matrix: [P, N_CHUNKS*512] bf16, w_host[p, t*512+n] = Wm[t, p, n]
    where Wm[t, gl*4+c, gl*16+d] = C[32t+gl, c, d]."""
    p4 = np.asarray(params, dtype=np.float32)            # [G, 4, D]
    p00, p01, p10, p11 = p4[:, 0], p4[:, 1], p4[:, 2], p4[:, 3]
    c = np.stack(
        [
            (p00 + p01 + p10 + p11) * 0.25,
            (p10 + p11 - p00 - p01) * 0.25,
            (p01 + p11 - p00 - p10) * 0.25,
            (p00 + p11 - p01 - p10) * 0.25,
        ],
        axis=1,
    )                                                    # [G, 4, D]
    wm = np.zeros((N_CHUNKS, P, CHUNK_COLS), np.float32)
    cr = c.reshape(N_CHUNKS, GPC, 4, OUT_DIM)
    for gl in range(GPC):
        wm[:, gl * 4:(gl + 1) * 4, gl * OUT_DIM:(gl + 1) * OUT_DIM] = cr[:, gl]
    w_host = np.ascontiguousarray(wm.transpose(1, 0, 2).reshape(P, N_CHUNKS * CHUNK_COLS))
    return w_host.astype(ml_dtypes.bfloat16)


_NC_CACHE = {}


def kernel(X: np.ndarray, params: np.ndarray) -> np.ndarray:
    X = np.ascontiguousarray(np.asarray(X, dtype=np.float32))
    assert X.shape == (BATCH, GROUPS * ARITY)
    w_host = make_w_host(params)

    if "nc" not in _NC_CACHE:
        _NC_CACHE["nc"] = build_nc()
    nc = _NC_CACHE["nc"]

    in_maps = [
        {"x": X[i * B_LOC:(i + 1) * B_LOC], "w": w_host} for i in range(N_CORES)
    ]
    res = run_bass_kernel_spmd(nc, in_maps, core_ids=list(range(N_CORES)))
    return np.concatenate(
        [np.asarray(res.results[i]["out"]).astype(np.float32) for i in range(N_CORES)],
        axis=0,
    )
